# revision 1
# baseline (speedup 1.0000x reference)
# Trainium2 Bass kernel for nn_LiteMultiscaleAttention (8-core data-parallel over batch).
#
# Per core (one batch):
#   qkv = Wqkv @ x                       -> bf16, padded [128,12,68,68] layout
#   agg = grouped-pw(dw5x5(qkv))         -> fused per-tap block-diag matmuls (25 taps, PSUM accum)
#   attention (64 heads, linear):        -> streamed PE transposes + per-8-head-group matmuls
#   y = BN(Wproj @ attn)                 -> bias folded on host
import sys
import os
import numpy as np

sys.path.insert(0, '/opt/trn_rl_repo')

import ml_dtypes
import concourse.bass as bass
import concourse.mybir as mybir
import concourse.tile as tile
from concourse import bacc
from concourse.bass_utils import run_bass_kernel_spmd
from concourse.masks import make_identity

BF16 = mybir.dt.bfloat16
F32 = mybir.dt.float32

B, CIN, H, W = 8, 512, 64, 64
S = H * W                 # 4096
C3 = 1536                 # qkv channels
NCT = 12                  # channel tiles of qkv/agg
NKT = 4                   # k-tiles of Cin
NNT = 8                   # 512-wide spatial chunks (8 image rows each)
NH = 64                   # total heads
EPS = 1e-15
BN_EPS = 1e-5

_CACHED = {}


def _head_src_block(h):
    """For head h (canonical: 0..31 qkv, 32..63 agg) return (is_agg, c0) with
    c0 the base channel of its q rows; k at c0+16, v at c0+32."""
    return (h >= 32, 48 * (h % 32))


def build_program(dbg=False):
    nc = bacc.Bacc('TRN2', target_bir_lowering=False, debug=False)

    # ---------------- DRAM I/O ----------------
    x_d = nc.dram_tensor('x_b', [CIN, S], F32, kind='ExternalInput')
    wq_d = nc.dram_tensor('wq', [128, NKT, C3], BF16, kind='ExternalInput')
    w2c_d = nc.dram_tensor('w2c', [128, 25, NCT, 32], BF16, kind='ExternalInput')
    wp_d = nc.dram_tensor('wp', [128, 8, 512], BF16, kind='ExternalInput')
    bnb_d = nc.dram_tensor('bnb', [128, 4], F32, kind='ExternalInput')
    obd_d = nc.dram_tensor('obd', [128, 64], BF16, kind='ExternalInput')
    y_d = nc.dram_tensor('y_b', [512, S], F32, kind='ExternalOutput')
    # DRAM scratch for the gathered q rows (relu applied on reload)
    qst_d = nc.dram_tensor('q_stack', [128, 8, S], BF16)
    if dbg:
        dbg_qkv = nc.dram_tensor('dbg_qkv', [128, NCT, 68, 68], BF16, kind='ExternalOutput')
        dbg_agg = nc.dram_tensor('dbg_agg', [128, NCT, S], BF16, kind='ExternalOutput')
        dbg_attn = nc.dram_tensor('dbg_attn', [128, 8, S], BF16, kind='ExternalOutput')
        dbg_bdn = nc.dram_tensor('dbg_bdn', [128, 8, 64], BF16, kind='ExternalOutput')
        dbg_den = nc.dram_tensor('dbg_den', [128, 8, 1], F32, kind='ExternalOutput')

    with tile.TileContext(nc) as tc:
        from contextlib import ExitStack
        ctx = ExitStack()
        with ctx:
            stat = ctx.enter_context(tc.tile_pool(name='stat', bufs=1))

            id128 = stat.tile([128, 128], BF16)
            make_identity(nc, id128[:])
            ones_col = stat.tile([128, 1], BF16)
            nc.gpsimd.memset(ones_col[:], 1.0)
            ones512 = stat.tile([1, 512], BF16)
            nc.gpsimd.memset(ones512[:], 1.0)
            epsw = stat.tile([1, 128], BF16)
            nc.gpsimd.memset(epsw[:], EPS)
            # block-diagonal ones: 16x16 ones blocks (head-local), used to
            # replicate den_col into rank-1-per-head den weights
            ones64 = stat.tile([128, 64], BF16)
            nc.sync.dma_start(ones64[:], obd_d.ap())

            # attention block-diag weights (built by stage A evacuations)
            bdn = stat.tile([128, 8, 64], BF16)      # per tg: 8x[16,16] vk blocks
            nc.gpsimd.memset(bdn[:], 0.0)
            bdd = stat.tile([128, 8, 64], BF16)      # rank-1 kones blocks
            nc.gpsimd.memset(bdd[:], 0.0)
            den_col = stat.tile([128, 8, 1], F32)

            # ---------------- Phase 0: load x, weights ----------------
            es_early = ExitStack()
            es_mid = ExitStack()
            psum = es_mid.enter_context(tc.tile_pool(name='psum', bufs=2, space='PSUM'))
            w2cp = es_mid.enter_context(tc.tile_pool(name='w2cp', bufs=1))
            w2c = w2cp.tile([128, 25, NCT, 32], BF16)
            nc.sync.dma_start(w2c[:], w2c_d.ap())
            qkvp = es_mid.enter_context(tc.tile_pool(name='qkvp', bufs=1))

            w1 = es_early.enter_context(tc.tile_pool(name='w1', bufs=1))
            wq = w1.tile([128, NKT, C3], BF16)
            nc.sync.dma_start(wq[:], wq_d.ap())
            x16p = es_early.enter_context(tc.tile_pool(name='x16p', bufs=1))
            x16 = x16p.tile([128, NKT, S], BF16)
            with tc.tile_pool(name='xf', bufs=2) as xfp:
                for kt in range(NKT):
                    xf = xfp.tile([128, S], F32, tag='xf')
                    nc.sync.dma_start(xf[:], x_d.ap()[128 * kt:128 * (kt + 1), :])
                    nc.vector.tensor_copy(x16[:, kt, :], xf[:])

            # ---------------- Phase 1: qkv (padded bf16) ----------------
            qkv = qkvp.tile([128, NCT, 68, 68], BF16)
            nc.gpsimd.memset(qkv[:], 0.0)
            for ct in range(NCT):
                for nt in range(NNT):
                    ps = psum.tile([128, 512], F32, tag='ps512', bufs=2)
                    for kt in range(NKT):
                        nc.tensor.matmul(
                            ps[:], wq[:, kt, 128 * ct:128 * (ct + 1)],
                            x16[:, kt, 512 * nt:512 * (nt + 1)],
                            start=(kt == 0), stop=(kt == NKT - 1))
                    dst = qkv[:, ct, 8 * nt + 2:8 * nt + 10, 2:66]
                    if nt % 2 == 0:
                        nc.vector.tensor_copy(dst, ps[:])
                    else:
                        nc.scalar.activation(dst, ps[:], mybir.ActivationFunctionType.Copy)

            es_early.close()  # frees wq / x16 space
            es_2 = ExitStack()
            trp = es_2.enter_context(tc.tile_pool(name='trp', bufs=4))
            holdp = es_2.enter_context(tc.tile_pool(name='holdp', bufs=2))

            # ---------------- helpers for attention stages ----------------
            def gather_block_from_qkv(dst_ap, c0):
                """DMA one 16-row interior block from padded qkv into dst."""
                src = qkv[c0 % 128:c0 % 128 + 16, c0 // 128, 2:66, 2:66]
                nc.sync.dma_start(dst_ap, src)

            def stage_a_for_group(tg, khold, vhold):
                """khold/vhold: [128, 4096] bf16 (k rows already relu'd).
                Accumulate vk^T for 8 heads into psA, then evacuate into bdn/bdd/den_col."""
                psA = psum.tile([128, 136], F32, tag='psA')
                psAd = psum.tile([128, 1], F32, tag='psAd')
                for st in range(32):
                    psT = psum.tile([128, 128], BF16, tag='tr')
                    kTt = trp.tile([128, 128], BF16, tag='kTt')
                    nc.tensor.transpose(psT[:], khold[:, 128 * st:128 * (st + 1)], id128[:])
                    if st % 2 == 0:
                        nc.vector.tensor_copy(kTt[:], psT[:])
                    else:
                        nc.scalar.activation(kTt[:], psT[:], mybir.ActivationFunctionType.Copy)
                    psT2 = psum.tile([128, 128], BF16, tag='tr')
                    vTt = trp.tile([128, 128], BF16, tag='vTt')
                    nc.tensor.transpose(psT2[:], vhold[:, 128 * st:128 * (st + 1)], id128[:])
                    if st % 2 == 1:
                        nc.vector.tensor_copy(vTt[:], psT2[:])
                    else:
                        nc.scalar.activation(vTt[:], psT2[:], mybir.ActivationFunctionType.Copy)
                    nc.tensor.matmul(psA[:, 0:128], kTt[:], vTt[:],
                                     start=(st == 0), stop=(st == 31))
                    nc.tensor.matmul(psAd[:], kTt[:], ones_col[:],
                                     start=(st == 0), stop=(st == 31))
                # evacuate diag blocks; PSUM partition access must be 32-aligned,
                # so copy head-pairs [32,32] masked by the block-diag ones pattern
                for j in range(4):
                    r0 = 32 * j
                    cc = 32 * (j % 2)
                    nc.vector.scalar_tensor_tensor(
                        bdn[r0:r0 + 32, tg, cc:cc + 32],
                        psA[r0:r0 + 32, r0:r0 + 32], 1.0,
                        ones64[r0:r0 + 32, cc:cc + 32],
                        mybir.AluOpType.mult, mybir.AluOpType.mult)
                nc.vector.tensor_copy(den_col[:, tg, :], psAd[:])
                # bdd blocks: ones64 * den_col  (per half)
                nc.vector.tensor_scalar_mul(bdd[0:64, tg, :], ones64[0:64, :], den_col[0:64, tg, :])
                nc.vector.tensor_scalar_mul(bdd[64:128, tg, :], ones64[64:128, :], den_col[64:128, tg, :])

            # ---------------- Phase 2a: qkv-head groups (tg 0..3) ----------------
            for tg in range(4):
                khold = holdp.tile([128, S], BF16, tag='kh')
                vhold = holdp.tile([128, S], BF16, tag='vh')
                for hl in range(8):
                    h = 8 * tg + hl
                    _, c0 = _head_src_block(h)
                    gather_block_from_qkv(khold[16 * hl:16 * hl + 16, :], c0 + 16)
                    gather_block_from_qkv(vhold[16 * hl:16 * hl + 16, :], c0 + 32)
                    gather_qs = qst_d.ap()[16 * hl:16 * hl + 16, tg, :]
                    src = qkv[(c0) % 128:(c0) % 128 + 16, c0 // 128, 2:66, 2:66]
                    nc.sync.dma_start(gather_qs, src)
                nc.vector.tensor_scalar_max(khold[:], khold[:], 0.0)
                stage_a_for_group(tg, khold, vhold)

            # ---------------- Phase 2b: fused dw+pw taps + agg-head groups ----------------
            es_2b = ExitStack()
            lhsp = es_2b.enter_context(tc.tile_pool(name='lhsp', bufs=2))
            astp = es_2b.enter_context(tc.tile_pool(name='astp', bufs=2))
            agg_khold = {}
            agg_vhold = {}
            for ct in range(NCT):
                # expand compact W2 -> block-diag lhsT for this channel tile
                lt = lhsp.tile([128, 25, 128], BF16, tag='lt')
                nc.gpsimd.memset(lt[:], 0.0)
                for blk in range(4):
                    nc.sync.dma_start(lt[32 * blk:32 * blk + 32, :, 32 * blk:32 * blk + 32],
                                      w2c[32 * blk:32 * blk + 32, :, ct, :])
                ast = astp.tile([128, S], BF16, tag='ast')
                for nt in range(NNT):
                    ps = psum.tile([128, 512], F32, tag='ps512', bufs=2)
                    for tap in range(25):
                        dy, dx = tap // 5, tap % 5
                        rhs = qkv[:, ct, 8 * nt + dy:8 * nt + dy + 8, dx:dx + 64]
                        nc.tensor.matmul(ps[:], lt[:, tap, :], rhs,
                                         start=(tap == 0), stop=(tap == 24))
                    dst = ast[:, 512 * nt:512 * (nt + 1)]
                    if nt % 2 == 0:
                        nc.vector.tensor_copy(dst, ps[:])
                    else:
                        nc.scalar.activation(dst, ps[:], mybir.ActivationFunctionType.Copy)
                # route this tile's 16-row blocks to their destinations
                for bi in range(8):
                    c = 128 * ct + 16 * bi
                    h = 32 + c // 48
                    r = c % 48
                    tg = h // 8
                    hl = h % 8
                    if tg not in agg_khold:
                        agg_khold[tg] = holdp.tile([128, S], BF16, tag='kh', name=f'aggkh{tg}')
                        agg_vhold[tg] = holdp.tile([128, S], BF16, tag='vh', name=f'aggvh{tg}')
                    if r == 0:
                        nc.sync.dma_start(qst_d.ap()[16 * hl:16 * hl + 16, tg, :],
                                          ast[16 * bi:16 * bi + 16, :])
                    elif r == 16:
                        nc.sync.dma_start(agg_khold[tg][16 * hl:16 * hl + 16, :],
                                          ast[16 * bi:16 * bi + 16, :])
                    else:
                        nc.sync.dma_start(agg_vhold[tg][16 * hl:16 * hl + 16, :],
                                          ast[16 * bi:16 * bi + 16, :])
                if dbg:
                    nc.sync.dma_start(dbg_agg.ap()[:, ct, :], ast[:])
                if ct % 3 == 2:
                    tg = 4 + ct // 3
                    nc.vector.tensor_scalar_max(agg_khold[tg][:], agg_khold[tg][:], 0.0)
                    stage_a_for_group(tg, agg_khold[tg], agg_vhold[tg])
                    del agg_khold[tg], agg_vhold[tg]

            if dbg:
                for ct in range(NCT):
                    nc.sync.dma_start(dbg_qkv.ap()[:, ct], qkv[:, ct])
            es_2b.close()
            es_2.close()
            es_mid.close()  # frees qkv / w2c / hold / staging space

            # ---------------- Phase 5: stage B ----------------
            psumB = ctx.enter_context(tc.tile_pool(name='psumB', bufs=4, space='PSUM'))
            attnp = ctx.enter_context(tc.tile_pool(name='attnp', bufs=1))
            attn = attnp.tile([128, 8, S], BF16)
            wpp = ctx.enter_context(tc.tile_pool(name='wpp', bufs=1))
            wp = wpp.tile([128, 8, 512], BF16)
            nc.sync.dma_start(wp[:], wp_d.ap())
            bnbp = ctx.enter_context(tc.tile_pool(name='bnbp', bufs=1))
            bnb = bnbp.tile([128, 4], F32)
            nc.sync.dma_start(bnb[:], bnb_d.ap())

            qbp = ctx.enter_context(tc.tile_pool(name='qbp', bufs=2))
            drp = ctx.enter_context(tc.tile_pool(name='drp', bufs=2))
            for tg in range(8):
                qb = qbp.tile([128, S], BF16, tag='qb')
                nc.sync.dma_start(qb[:], qst_d.ap()[:, tg, :])
                nc.vector.tensor_scalar_max(qb[:], qb[:], 0.0)
                for nt in range(NNT):
                    sl = slice(512 * nt, 512 * (nt + 1))
                    psN = psumB.tile([128, 512], F32, tag='psN', bufs=2)
                    nc.tensor.matmul(psN[0:64, :], bdn[0:64, tg, :], qb[0:64, sl],
                                     start=True, stop=True)
                    nc.tensor.matmul(psN[64:128, :], bdn[64:128, tg, :], qb[64:128, sl],
                                     start=True, stop=True)
                    psD = psumB.tile([128, 512], F32, tag='psD', bufs=2)
                    nc.tensor.matmul(psD[:], epsw[:], ones512[:], start=True, stop=False)
                    nc.tensor.matmul(psD[0:64, :], bdd[0:64, tg, :], qb[0:64, sl],
                                     start=False, stop=True)
                    nc.tensor.matmul(psD[64:128, :], bdd[64:128, tg, :], qb[64:128, sl],
                                     start=False, stop=True)
                    drt = drp.tile([128, 512], F32, tag='drt')
                    nc.vector.reciprocal_approx_fast(drt[:], psD[:])
                    nc.vector.scalar_tensor_tensor(
                        attn[:, tg, sl], psN[:], 1.0, drt[:],
                        mybir.AluOpType.mult, mybir.AluOpType.mult)

            if dbg:
                nc.sync.dma_start(dbg_attn.ap(), attn[:])
                nc.sync.dma_start(dbg_bdn.ap(), bdn[:])
                nc.sync.dma_start(dbg_den.ap(), den_col[:])

            # ---------------- Phase 6: proj + BN ----------------
            with tc.tile_pool(name='ysp', bufs=3) as ysp:
                for mt in range(4):
                    for nt in range(NNT):
                        ps = psumB.tile([128, 512], F32, tag='psN', bufs=2)
                        for kt in range(8):
                            nc.tensor.matmul(ps[:], wp[:, kt, 128 * mt:128 * (mt + 1)],
                                             attn[:, kt, 512 * nt:512 * (nt + 1)],
                                             start=(kt == 0), stop=(kt == 7))
                        ys = ysp.tile([128, 512], F32, tag='ys')
                        if nt % 2 == 0:
                            nc.vector.tensor_scalar_add(ys[:], ps[:], bnb[:, mt:mt + 1])
                        else:
                            nc.scalar.activation(ys[:], ps[:],
                                                 mybir.ActivationFunctionType.Identity,
                                                 bias=bnb[:, mt:mt + 1])
                        nc.sync.dma_start(
                            y_d.ap()[128 * mt:128 * (mt + 1), 512 * nt:512 * (nt + 1)], ys[:])

    nc.compile()
    return nc


def host_weights(w_qkv, w_dw, w_pw, w_proj, bn_gamma, bn_beta, bn_mean, bn_var):
    wq = w_qkv[:, :, 0, 0].astype(np.float32)       # [1536, 512]
    wdw = w_dw[:, 0].reshape(1536, 25).astype(np.float32)
    wpw = w_pw[:, :, 0, 0].astype(np.float32)       # [1536, 32]
    A = wdw.reshape(12, 4, 32, 25)
    Bm = wpw.reshape(12, 4, 32, 32)
    W2c = A[:, :, :, :, None] * Bm.transpose(0, 1, 3, 2)[:, :, :, None, :]
    W2c = np.ascontiguousarray(W2c.transpose(1, 2, 3, 0, 4).reshape(128, 25, 12, 32))
    inv = bn_gamma / np.sqrt(bn_var + BN_EPS)
    wp_f = (w_proj[:, :, 0, 0] * inv[:, None]).T    # [1024, 512] lhsT
    bnb = (bn_beta - bn_mean * inv).astype(np.float32)

    wq_dev = np.ascontiguousarray(
        wq.T.reshape(NKT, 128, C3).transpose(1, 0, 2)).astype(ml_dtypes.bfloat16)
    w2c_dev = W2c.astype(ml_dtypes.bfloat16)
    wp_dev = np.ascontiguousarray(
        wp_f.reshape(8, 128, 512).transpose(1, 0, 2)).astype(ml_dtypes.bfloat16)
    bnb_dev = np.ascontiguousarray(bnb.reshape(4, 128).T).astype(np.float32)
    obd = np.zeros((128, 64), np.float32)
    for half in (0, 64):
        for j in range(4):
            obd[half + 16 * j:half + 16 * j + 16, 16 * j:16 * j + 16] = 1.0
    obd_dev = obd.astype(ml_dtypes.bfloat16)
    return {'wq': wq_dev, 'w2c': w2c_dev, 'wp': wp_dev, 'bnb': bnb_dev, 'obd': obd_dev}


def kernel(x, w_qkv, w_dw, w_pw, w_proj, bn_gamma, bn_beta, bn_mean, bn_var):
    x = np.asarray(x, dtype=np.float32)
    wdev = host_weights(
        np.asarray(w_qkv, np.float32), np.asarray(w_dw, np.float32),
        np.asarray(w_pw, np.float32), np.asarray(w_proj, np.float32),
        np.asarray(bn_gamma, np.float32), np.asarray(bn_beta, np.float32),
        np.asarray(bn_mean, np.float32), np.asarray(bn_var, np.float32))

    if 'nc' not in _CACHED:
        _CACHED['nc'] = build_program()
    nc = _CACHED['nc']

    in_maps = []
    for b in range(B):
        in_maps.append({'x_b': np.ascontiguousarray(x[b].reshape(CIN, S)), **wdev})
    res = run_bass_kernel_spmd(nc, in_maps, list(range(B)))
    y = np.stack([res.results[b]['y_b'].reshape(512, H, W) for b in range(B)])
    return y.astype(np.float32)

